# revision 10
# baseline (speedup 1.0000x reference)
"""CommonNeighborsPredictor kernel for 8 Trainium2 NeuronCores.

Math (see reference):
    deg = adj.sum(-1) + 1e-6
    x   = emb + (adj @ emb) / deg[:, None]
    xn  = x / max(||x||_2, 1e-8)                            # row-normalize
    w_e = sum_c adj[src_e, c] * adj[dst_e, c] * (xn[src_e]@xn[c]) * (xn[dst_e]@xn[c])
    out = sigmoid(w)

Distribution (2 SPMD launches, no collectives):
  Stage 1: shard nodes (rows of adj) 8 ways; each core computes xn^T for its
    1250 nodes.  The adjacency is fed as fp8_e4m3 (0/1 values are exact) in a
    DoubleRow-interleaved layout so the PE contracts K=256 per matmul at 2x
    fp8 rate.  Degrees ride the PE as an extra M=1 ones-matmul for columns
    [0:1024] (PSUM bank budget) and a small DVE+gpsimd fold for [1024:1250].
    The normalization epilogue uses activation-engine Dsqrt for 1/deg and
    1/||x|| (Square(Dsqrt(x/4)) == 1/x) and gpsimd partition_broadcast
    instead of PE broadcast matmuls.
  Stage 2: shard query edges 8 ways (512 each).  Adjacency rows for the two
    endpoints are indirect-DMA-gathered in fp8 from a per-core dedup'd table;
    gpsimd computes the common-neighbor mask cn = aS*aD (bf16 out); the two
    cos matrices come from DoubleRow fp8 matmuls against resident interleaved
    xn; the scalar engine copies cosR out of PSUM to bf16 so the DVE mask
    multiply runs at 2x, and a fused scalar_tensor_tensor does the final
    product + row-reduction.  Sigmoid on the scalar engine.

dtypes: adjacency and xn are fp8_e4m3 (adjacency exact; xn rounding gives
~1e-3 max output error vs the fp32 reference).  PSUM accumulation and the
normalization epilogue are fp32; masks/products bf16.
"""

import numpy as np

import concourse.bass as bass
import concourse.bacc as bacc
import concourse.mybir as mybir
import concourse.tile as tile
from concourse import bass_utils

F32 = mybir.dt.float32
BF16 = mybir.dt.bfloat16
F8 = mybir.dt.float8e4
I32 = mybir.dt.int32
AF = mybir.ActivationFunctionType
OP = mybir.AluOpType
DR = mybir.MatmulPerfMode.DoubleRow
NP_BF16 = mybir.dt.np(BF16)
NP_F8 = mybir.dt.np(F8)

N, D, Q, NC = 10000, 256, 4096, 8
MSH = N // NC            # 1250 nodes per core (stage 1)
KSUP = 40                # k super-tiles of 256 rows (10240 padded)
KP2 = KSUP * 256
GS = 4                   # super-tiles per DMA group (1.28 MB each)
NG = KSUP // GS
QL = Q // NC             # 512 edges per core (stage 2)
ETW = 128                # edges per tile
NET = QL // ETW
PEDEG = 1024             # deg columns computed on PE (bank budget); rest on DVE
PREF = 6                 # stage1 adj group DMAs in flight
CLS_B_EVERY = 5          # stage2: every 5th chunk offloads PSUM reads to ACT + products to GP


def _chunks(total, step):
    return [(s, min(step, total - s)) for s in range(0, total, step)]


def build_stage1(nc_cores=NC):
    """Per-core: xnT shard [256, 1250] fp8 from DoubleRow-packed adjT + emb."""
    msh = MSH
    b = bacc.Bacc("TRN2", target_bir_lowering=False, debug=False, num_devices=nc_cores)
    # (p, g*GS*2*msh + s*2*msh + j*msh + m) = adjT[(g*GS+s)*256 + j*128 + p, m]
    adjx = b.dram_tensor("adjx", [128, KSUP * 2 * msh], F8, kind="ExternalInput")
    # (p, st*512 + j*256 + d) = emb[st*256 + j*128 + p, d]
    embx = b.dram_tensor("embx", [128, KSUP * 512], F8, kind="ExternalInput")
    embT = b.dram_tensor("embT", [D, msh], BF16, kind="ExternalInput")
    xnT = b.dram_tensor("xnT", [D, msh], F8, kind="ExternalOutput")

    ychunks = [(0, 512), (512, 512), (1024, msh - 1024)]
    dchunks = [(0, 512), (512, 512)]  # PE-deg columns
    dvw = msh - PEDEG                 # DVE-deg columns (226)

    with tile.TileContext(b) as tc:
        with (
            tc.tile_pool(name="const", bufs=1) as cpool,
            tc.tile_pool(name="stream", bufs=PREF) as spool,
            tc.tile_pool(name="work", bufs=2) as wpool,
            tc.tile_pool(name="acc", bufs=1, space="PSUM") as apool,
            tc.tile_pool(name="degp", bufs=1, space="PSUM") as dpool,
        ):
            embt = cpool.tile([128, KSUP * 512], F8, name="embt")
            EW = KSUP * 512 // 4

            def embt_dma(i):
                b.sync.dma_start(out=embt[:, i * EW : (i + 1) * EW],
                                 in_=embx.ap()[:, i * EW : (i + 1) * EW])

            grp = {}

            def grp_dma(g):
                t = spool.tile([128, GS * 2 * msh], F8, tag="adjg", name=f"adjg{g}")
                # two half-transfers to spread across more DMA queue rows
                h = GS * msh  # half the group columns
                base = g * 2 * GS * msh
                b.sync.dma_start(out=t[:, :h], in_=adjx.ap()[:, base : base + h])
                b.sync.dma_start(out=t[:, h:], in_=adjx.ap()[:, base + h : base + 2 * h])
                grp[g] = t

            embt_dma(0)
            for g in range(min(PREF, NG)):
                grp_dma(g)
            for i in range(1, 4):
                embt_dma(i)

            embT_sb = []
            for dh in range(2):
                t = cpool.tile([128, msh], BF16, name=f"embT{dh}")
                b.sync.dma_start(out=t[:], in_=embT.ap()[dh * 128 : (dh + 1) * 128, :])
                embT_sb.append(t)

            ones2 = cpool.tile([128, 32], F8, name="ones2")
            b.vector.memset(ones2[:], 1.0)
            ones_col = cpool.tile([128, 1], BF16, name="onescol")
            b.vector.memset(ones_col[:, :1], 1.0)
            bias_deg = cpool.tile([1, 1], F32, name="biasdeg")
            b.vector.memset(bias_deg[:1, :1], 1e-6)
            bias_ns = cpool.tile([1, 1], F32, name="biasns")
            b.vector.memset(bias_ns[:1, :1], 1e-16)

            ps_y = {
                (dh, c0): apool.tile(
                    [128, cw], F32, tag=f"py{dh}_{c0}", name=f"py{dh}_{c0}"
                )
                for dh in range(2)
                for (c0, cw) in ychunks
            }
            ps_d = {
                c0: dpool.tile([1, cw], F32, tag=f"pd{c0}", name=f"pd{c0}")
                for (c0, cw) in dchunks
            }
            # DVE-deg tail accumulators (2 chains so adds pipeline)
            dtail = [cpool.tile([128, 2 * dvw], BF16, name=f"dt{j}") for j in range(2)]

            for st in range(KSUP):
                g, s = divmod(st, GS)
                if g not in grp:
                    grp_dma(g)
                at = grp[g]
                base = s * 2 * msh
                at3 = at[:, base : base + 2 * msh].rearrange("p (j m) -> p j m", j=2)
                emb3 = embt[:, st * 512 : (st + 1) * 512].rearrange(
                    "p (j d) -> p j d", j=2
                )
                first, last = (st == 0), (st == KSUP - 1)
                for dh in range(2):
                    lhsT = emb3[:, :, dh * 128 : (dh + 1) * 128]
                    for (c0, cw) in ychunks:
                        b.tensor.matmul(
                            ps_y[(dh, c0)][:],
                            lhsT=lhsT,
                            rhs=at3[:, :, c0 : c0 + cw],
                            start=first,
                            stop=last,
                            perf_mode=DR,
                        )
                ones3 = ones2[:, :].rearrange("p (j o) -> p j o", j=2)
                for (c0, cw) in dchunks:
                    b.tensor.matmul(
                        ps_d[c0][:1, :],
                        lhsT=ones3[:, :, :1],
                        rhs=at3[:, :, c0 : c0 + cw],
                        start=first,
                        stop=last,
                        perf_mode=DR,
                    )
                # deg tail [PEDEG:msh] on DVE: fold both j-halves as columns
                tl = at[:, base + PEDEG : base + msh]
                th = at[:, base + msh + PEDEG : base + 2 * msh]
                j = st % 2
                if st < 2:
                    b.vector.tensor_copy(dtail[j][:, :dvw], tl)
                    b.vector.tensor_copy(dtail[j][:, dvw:], th)
                else:
                    b.vector.tensor_add(dtail[j][:, :dvw], dtail[j][:, :dvw], tl)
                    b.vector.tensor_add(dtail[j][:, dvw:], dtail[j][:, dvw:], th)
                if s == 0 and g + PREF < NG and (g + PREF) not in grp:
                    grp_dma(g + PREF)
                if last:
                    for gg in list(grp):
                        grp.pop(gg)

            # ---- deg tail: fold chains + halves, partition-reduce on gpsimd
            b.vector.tensor_add(dtail[0][:], dtail[0][:], dtail[1][:])
            dt_f = wpool.tile([128, dvw], BF16, tag="dtf")
            b.vector.tensor_add(dt_f[:], dtail[0][:, :dvw], dtail[0][:, dvw:])
            dt_r = wpool.tile([128, dvw], BF16, tag="dtr")
            b.gpsimd.partition_all_reduce(
                dt_r[:], dt_f[:], channels=128, reduce_op=bass.bass_isa.ReduceOp.add
            )

            # ---- epilogue: rinv = 1/(deg+1e-6) = exp(-ln(deg+1e-6)) on ACT
            rinv = wpool.tile([1, msh], F32, tag="rinv")
            for (c0, cw) in dchunks:
                b.scalar.activation(
                    rinv[:1, c0 : c0 + cw], ps_d[c0][:1, :], AF.Ln,
                    bias=bias_deg[:1, :1],
                )
            b.scalar.activation(
                rinv[:1, PEDEG:msh], dt_r[:1, :], AF.Ln, bias=bias_deg[:1, :1]
            )
            b.scalar.activation(rinv[:1, :], rinv[:1, :], AF.Exp, scale=-1.0)
            rinv_b = wpool.tile([128, msh], F32, tag="rinvb")
            b.gpsimd.partition_broadcast(rinv_b[:], rinv[:1, :], channels=128)

            xts = []
            sqs = []
            for dh in range(2):
                xt = wpool.tile([128, msh], F32, tag=f"xt{dh}")
                for (c0, cw) in ychunks:
                    b.vector.tensor_mul(
                        xt[:, c0 : c0 + cw], ps_y[(dh, c0)][:], rinv_b[:, c0 : c0 + cw]
                    )
                b.vector.tensor_add(xt[:], xt[:], embT_sb[dh][:])
                xts.append(xt)
                sq = wpool.tile([128, msh], BF16, tag=f"sq{dh}")
                b.scalar.activation(sq[:], xt[:], AF.Square)
                sqs.append(sq)
            # norm^2 column sums in 512-chunks reusing the freed deg banks
            rn = wpool.tile([1, msh], F32, tag="rn")
            for (c0, cw) in ychunks:
                ns_c = dpool.tile([1, 512], F32, tag=f"pd{c0 % 1024}", name=f"ns{c0}")
                for dh in range(2):
                    b.tensor.matmul(
                        ns_c[:1, :cw], lhsT=ones_col[:, :1],
                        rhs=sqs[dh][:, c0 : c0 + cw],
                        start=(dh == 0), stop=(dh == 1),
                    )
                b.scalar.activation(
                    rn[:1, c0 : c0 + cw], ns_c[:1, :cw], AF.Ln,
                    bias=bias_ns[:1, :1],
                )
            b.scalar.activation(rn[:1, :], rn[:1, :], AF.Exp, scale=-0.5)
            rn_b = wpool.tile([128, msh], F32, tag="rnb")
            b.gpsimd.partition_broadcast(rn_b[:], rn[:1, :], channels=128)
            for dh in range(2):
                xn = wpool.tile([128, msh], F8, tag=f"xn{dh}")
                b.vector.tensor_mul(xn[:], xts[dh][:], rn_b[:])
                b.sync.dma_start(
                    out=xnT.ap()[dh * 128 : (dh + 1) * 128, :], in_=xn[:]
                )
    b.compile()
    return b


def build_stage2(nc_cores=NC):
    """Per-core: w [512, 1].

    sumbuf_et = gather(tblW, src) then += gather(tblS slabs, dst) via SWDGE
    compute_op=add (descriptors must be <=2048B on the compute path, hence the
    5x2000 slab table).  Values {0,1,2}; common neighbor <=> sum >= 1.5.
    Per 1024-wide super-chunk (2 PSUM banks), three pipelines rotate:
      A (DVE):    t = (sum>=1.5)*cosL            [stt is_ge from PSUM]
      D (ACT+DVE): cnm=relu(sum-1); cosLs=copy;  t = cnm*cosLs  [DVE tt bf16 2x]
      G (ACT+GP):  same but GP does the multiply
    then always: parts[:,sc] = rowsum(t * cosR)  [DVE stt from PSUM + accum]
    """
    ql, etw, net = QL, ETW, NET
    r = 2 * ql
    SCW = 1024
    scs = _chunks(N, SCW)           # 9x1024 + 784
    nsc = len(scs)
    SLW = 2000                      # add-gather slab width (<=2048B)
    NSLAB = N // SLW

    b = bacc.Bacc(
        "TRN2",
        target_bir_lowering=False,
        debug=False,
        num_devices=nc_cores,
        dynamic_dma_scratch_size=65536,
    )
    # (p, j*N + c) = xn[j*128 + p, c]
    xns = b.dram_tensor("xns", [128, 2 * N], F8, kind="ExternalInput")
    tblW = b.dram_tensor("tblW", [r, N], F8, kind="ExternalInput")
    tblS = b.dram_tensor("tblS", [NSLAB, r, SLW], F8, kind="ExternalInput")
    idxs = b.dram_tensor("idxs", [ql, 1], I32, kind="ExternalInput")
    idxd = b.dram_tensor("idxd", [ql, 1], I32, kind="ExternalInput")
    # (p, j*ql + e) = xn[j*128 + p, endpoint_e]
    ut = b.dram_tensor("ut", [128, 2 * ql], F8, kind="ExternalInput")
    vt = b.dram_tensor("vt", [128, 2 * ql], F8, kind="ExternalInput")
    w = b.dram_tensor("w", [ql, 1], F32, kind="ExternalOutput")

    def pattern(sci, et):
        m = sci % 6
        if m == 0:
            return "A"
        if m in (4, 5) and et >= 2:
            return "G"
        return "D"

    with tile.TileContext(b) as tc:
        with (
            tc.tile_pool(name="const", bufs=1) as cpool,
            tc.tile_pool(name="gather", bufs=4) as gpool,
            tc.tile_pool(name="mid", bufs=4) as dpool,
            tc.tile_pool(name="small", bufs=2) as wpool,
            tc.tile_pool(name="cos", bufs=2, space="PSUM") as ppool,
        ):
            ix_s, ix_d = [], []
            for et in range(net):
                ts_ = cpool.tile([etw, 1], I32, tag=f"ixs{et}")
                b.sync.dma_start(out=ts_[:], in_=idxs.ap()[et * etw : (et + 1) * etw, :1])
                ix_s.append(ts_)
                td_ = cpool.tile([etw, 1], I32, tag=f"ixd{et}")
                b.sync.dma_start(out=td_[:], in_=idxd.ap()[et * etw : (et + 1) * etw, :1])
                ix_d.append(td_)

            # gathers first: sumbuf = adj[src] (wide) += adj[dst] (slab chunks)
            pend = {}
            for et in range(net):
                sb_ = gpool.tile([etw, N], F8, tag="sb", bufs=4, name=f"sb{et}")
                b.gpsimd.indirect_dma_start(
                    out=sb_[:], out_offset=None, in_=tblW.ap(),
                    in_offset=bass.IndirectOffsetOnAxis(ap=ix_s[et][:, :1], axis=0),
                )
                for k in range(NSLAB):
                    b.gpsimd.indirect_dma_start(
                        out=sb_[:, k * SLW : (k + 1) * SLW], out_offset=None,
                        in_=tblS.ap()[0],
                        in_offset=bass.IndirectOffsetOnAxis(ap=ix_d[et][:, :1], axis=0),
                        compute_op=OP.add, element_offset=k * r * SLW,
                    )
                pend[et] = sb_

            # resident xn + endpoint tiles
            xnt = cpool.tile([128, 2 * N], F8, name="xnt")
            for i in range(4):
                cw = 2 * N // 4
                b.sync.dma_start(out=xnt[:, i * cw : (i + 1) * cw],
                                 in_=xns.ap()[:, i * cw : (i + 1) * cw])
            biasm1 = cpool.tile([128, 1], F32, name="biasm1")
            b.vector.memset(biasm1[:, :1], -1.0)
            utt = cpool.tile([128, 2 * ql], F8, name="utt")
            b.sync.dma_start(out=utt[:], in_=ut.ap()[:, :])
            vtt = cpool.tile([128, 2 * ql], F8, name="vtt")
            b.sync.dma_start(out=vtt[:], in_=vt.ap()[:, :])

            xn3 = xnt[:, :].rearrange("p (j c) -> p j c", j=2)
            ut3 = utt[:, :].rearrange("p (j e) -> p j e", j=2)
            vt3 = vtt[:, :].rearrange("p (j e) -> p j e", j=2)

            # alpha-prefill: tile 0's first ALPHA sc compute P = cosL*cosR
            # during the gather-descriptor window (no mask needed yet)
            ALPHA = 8
            ptiles = [cpool.tile([etw, SCW], BF16, name=f"pp{i}") for i in range(ALPHA)]

            sci = 0
            for et in range(net):
                esl = slice(et * etw, (et + 1) * etw)
                sb_ = pend.pop(et)
                parts = wpool.tile([etw, nsc], F32, tag="parts")
                if et == 0:
                    for ci in range(ALPHA):
                        c0, cw = scs[ci]
                        cosL = ppool.tile([etw, SCW], F32, tag="cosL")
                        cosR = ppool.tile([etw, SCW], F32, tag="cosR")
                        for h0, hw in _chunks(cw, 512):
                            b.tensor.matmul(
                                cosL[:, h0 : h0 + hw], lhsT=ut3[:, :, esl],
                                rhs=xn3[:, :, c0 + h0 : c0 + h0 + hw],
                                start=True, stop=True, perf_mode=DR,
                            )
                        for h0, hw in _chunks(cw, 512):
                            b.tensor.matmul(
                                cosR[:, h0 : h0 + hw], lhsT=vt3[:, :, esl],
                                rhs=xn3[:, :, c0 + h0 : c0 + h0 + hw],
                                start=True, stop=True, perf_mode=DR,
                            )
                        cosRs = dpool.tile([etw, SCW], BF16, tag="cosRs")
                        b.scalar.activation(cosRs[:, :cw], cosR[:, :cw], AF.Copy)
                        b.vector.tensor_mul(ptiles[ci][:, :cw], cosL[:, :cw],
                                            cosRs[:, :cw])
                    for ci in range(ALPHA):
                        c0, cw = scs[ci]
                        scr = dpool.tile([etw, SCW], BF16, tag="scr")
                        b.vector.scalar_tensor_tensor(
                            out=scr[:, :cw], in0=sb_[:, c0 : c0 + cw],
                            scalar=1.5, in1=ptiles[ci][:, :cw],
                            op0=OP.is_ge, op1=OP.mult,
                            accum_out=parts[:, ci : ci + 1],
                        )
                        sci += 1
                for ci, (c0, cw) in enumerate(scs):
                    if et == 0 and ci < ALPHA:
                        continue
                    cosL = ppool.tile([etw, SCW], F32, tag="cosL")
                    cosR = ppool.tile([etw, SCW], F32, tag="cosR")
                    for h0, hw in _chunks(cw, 512):
                        b.tensor.matmul(
                            cosL[:, h0 : h0 + hw], lhsT=ut3[:, :, esl],
                            rhs=xn3[:, :, c0 + h0 : c0 + h0 + hw],
                            start=True, stop=True, perf_mode=DR,
                        )
                    for h0, hw in _chunks(cw, 512):
                        b.tensor.matmul(
                            cosR[:, h0 : h0 + hw], lhsT=vt3[:, :, esl],
                            rhs=xn3[:, :, c0 + h0 : c0 + h0 + hw],
                            start=True, stop=True, perf_mode=DR,
                        )
                    t_ = dpool.tile([etw, SCW], BF16, tag="t_")
                    pat = pattern(sci, et)
                    if pat == "A":
                        b.vector.scalar_tensor_tensor(
                            out=t_[:, :cw], in0=sb_[:, c0 : c0 + cw],
                            scalar=1.5, in1=cosL[:, :cw],
                            op0=OP.is_ge, op1=OP.mult,
                        )
                    else:
                        cosLs = dpool.tile([etw, SCW], BF16, tag="cosLs")
                        b.scalar.activation(cosLs[:, :cw], cosL[:, :cw], AF.Copy)
                        cnm = dpool.tile([etw, SCW], BF16, tag="cnm")
                        b.scalar.activation(
                            cnm[:, :cw], sb_[:, c0 : c0 + cw], AF.Relu,
                            bias=biasm1[:, :1],
                        )
                        if pat == "G":
                            b.gpsimd.tensor_mul(t_[:, :cw], cnm[:, :cw], cosLs[:, :cw])
                        else:
                            b.vector.tensor_mul(t_[:, :cw], cnm[:, :cw], cosLs[:, :cw])
                    scr = dpool.tile([etw, SCW], BF16, tag="scr")
                    b.vector.scalar_tensor_tensor(
                        out=scr[:, :cw],
                        in0=t_[:, :cw],
                        scalar=1.0,
                        in1=cosR[:, :cw],
                        op0=OP.mult,
                        op1=OP.mult,
                        accum_out=parts[:, ci : ci + 1],
                    )
                    sci += 1
                wacc = wpool.tile([etw, 1], F32, tag="wacc")
                b.vector.reduce_sum(wacc[:, :1], parts[:], axis=mybir.AxisListType.X)
                sg = wpool.tile([etw, 1], F32, tag="sg")
                b.scalar.activation(sg[:, :1], wacc[:, :1], AF.Sigmoid)
                b.sync.dma_start(out=w.ap()[et * etw : (et + 1) * etw, :1], in_=sg[:, :1])
    b.compile()
    return b


def make_stage1_inputs(emb, adj, nc_cores=NC):
    msh = MSH
    e_pad = np.zeros((KP2, D), np.float32)
    e_pad[:N] = emb
    # (p, st*512 + j*256 + d) = emb[st*256 + j*128 + p, d]
    embx = np.ascontiguousarray(
        e_pad.reshape(KSUP, 2, 128, D).transpose(2, 0, 1, 3).reshape(128, KSUP * 512)
    ).astype(NP_F8)
    ins = []
    for k in range(nc_cores):
        sh = adj[k * msh : (k + 1) * msh, :]  # [msh, N] rows of this core
        adjT = np.zeros((KP2, msh), np.float32)
        adjT[:N] = sh.T
        # (p, st*2*msh + j*msh + m) = adjT[st*256 + j*128 + p, m]
        adjx = np.ascontiguousarray(
            adjT.reshape(KSUP, 2, 128, msh).transpose(2, 0, 1, 3).reshape(128, KSUP * 2 * msh)
        ).astype(NP_F8)
        embT = np.ascontiguousarray(emb[k * msh : (k + 1) * msh, :].T).astype(NP_BF16)
        ins.append({"adjx": adjx, "embx": embx, "embT": embT})
    return ins


def make_stage2_inputs(adj, xnT, src, dst_, nc_cores=NC):
    ql = QL
    # xns: (p, j*N + c) = xnT[j*128 + p, c]
    xns = np.ascontiguousarray(xnT.reshape(2, 128, N).transpose(1, 0, 2).reshape(128, 2 * N))
    xnT_f32 = xnT.astype(np.float32)
    ins = []
    for k in range(nc_cores):
        s_k = src[k * ql : (k + 1) * ql]
        d_k = dst_[k * ql : (k + 1) * ql]
        uniq = np.unique(np.concatenate([s_k, d_k]))
        tblW = np.zeros((2 * ql, N), NP_F8)
        tblW[: len(uniq)] = adj[uniq].astype(NP_F8)
        SLW = 2000
        # [NSLAB, r, SLW] column-slab layout for the accumulating gathers
        tblS = np.ascontiguousarray(
            tblW.reshape(2 * ql, N // SLW, SLW).transpose(1, 0, 2)
        )

        def pack_cols(cols):
            # [256, ql] -> (p, j*ql + e)
            g = xnT_f32[:, cols]
            return np.ascontiguousarray(
                g.reshape(2, 128, ql).transpose(1, 0, 2).reshape(128, 2 * ql)
            ).astype(NP_F8)

        ins.append(
            {
                "xns": xns,
                "tblW": tblW,
                "tblS": tblS,
                "idxs": np.searchsorted(uniq, s_k).astype(np.int32)[:, None],
                "idxd": np.searchsorted(uniq, d_k).astype(np.int32)[:, None],
                "ut": pack_cols(s_k),
                "vt": pack_cols(d_k),
            }
        )
    return ins


_progs = {}
LAST_RESULTS = []  # BassKernelResults of the most recent kernel() call (for profiling)


def _get(name, builder):
    if name not in _progs:
        _progs[name] = builder()
    return _progs[name]


def kernel(emb_weight, adj, edges):
    emb = np.asarray(emb_weight, dtype=np.float32)
    adj = np.asarray(adj, dtype=np.float32)
    edges = np.asarray(edges)
    src = edges[0].astype(np.int64)
    dst_ = edges[1].astype(np.int64)

    s1 = _get("s1", build_stage1)
    s2 = _get("s2", build_stage2)

    in1 = make_stage1_inputs(emb, adj)
    r1 = bass_utils.run_bass_kernel_spmd(s1, in1, core_ids=list(range(NC)))
    xnT = np.concatenate(
        [np.asarray(r1.results[k]["xnT"]) for k in range(NC)], axis=1
    )

    in2 = make_stage2_inputs(adj, xnT, src, dst_)
    r2 = bass_utils.run_bass_kernel_spmd(s2, in2, core_ids=list(range(NC)))
    w = np.concatenate([r2.results[k]["w"][:, 0] for k in range(NC)])

    LAST_RESULTS.clear()
    LAST_RESULTS.extend([r1, r2])
    return w.astype(np.float32)



# revision 11
# speedup vs baseline: 1.0001x; 1.0001x over previous
"""CommonNeighborsPredictor kernel for 8 Trainium2 NeuronCores.

Math (see reference):
    deg = adj.sum(-1) + 1e-6
    x   = emb + (adj @ emb) / deg[:, None]
    xn  = x / max(||x||_2, 1e-8)                            # row-normalize
    w_e = sum_c adj[src_e, c] * adj[dst_e, c] * (xn[src_e]@xn[c]) * (xn[dst_e]@xn[c])
    out = sigmoid(w)

Distribution (2 SPMD launches, no collectives):
  Stage 1: shard nodes (rows of adj) 8 ways; each core computes xn^T for its
    1250 nodes.  The adjacency is fed as fp8_e4m3 (0/1 values are exact) in a
    DoubleRow-interleaved layout so the PE contracts K=256 per matmul at 2x
    fp8 rate.  Degrees ride the PE as an extra M=1 ones-matmul for columns
    [0:1024] (PSUM bank budget) and a small DVE+gpsimd fold for [1024:1250].
    The normalization epilogue uses activation-engine Dsqrt for 1/deg and
    1/||x|| (Square(Dsqrt(x/4)) == 1/x) and gpsimd partition_broadcast
    instead of PE broadcast matmuls.
  Stage 2: shard query edges 8 ways (512 each).  Adjacency rows for the two
    endpoints are indirect-DMA-gathered in fp8 from a per-core dedup'd table;
    gpsimd computes the common-neighbor mask cn = aS*aD (bf16 out); the two
    cos matrices come from DoubleRow fp8 matmuls against resident interleaved
    xn; the scalar engine copies cosR out of PSUM to bf16 so the DVE mask
    multiply runs at 2x, and a fused scalar_tensor_tensor does the final
    product + row-reduction.  Sigmoid on the scalar engine.

dtypes: adjacency and xn are fp8_e4m3 (adjacency exact; xn rounding gives
~1e-3 max output error vs the fp32 reference).  PSUM accumulation and the
normalization epilogue are fp32; masks/products bf16.
"""

import numpy as np

import concourse.bass as bass
import concourse.bacc as bacc
import concourse.mybir as mybir
import concourse.tile as tile
from concourse import bass_utils

F32 = mybir.dt.float32
BF16 = mybir.dt.bfloat16
F8 = mybir.dt.float8e4
I32 = mybir.dt.int32
AF = mybir.ActivationFunctionType
OP = mybir.AluOpType
DR = mybir.MatmulPerfMode.DoubleRow
NP_BF16 = mybir.dt.np(BF16)
NP_F8 = mybir.dt.np(F8)

N, D, Q, NC = 10000, 256, 4096, 8
MSH = N // NC            # 1250 nodes per core (stage 1)
KSUP = 40                # k super-tiles of 256 rows (10240 padded)
KP2 = KSUP * 256
GS = 4                   # super-tiles per DMA group (1.28 MB each)
NG = KSUP // GS
QL = Q // NC             # 512 edges per core (stage 2)
ETW = 128                # edges per tile
NET = QL // ETW
PEDEG = 1024             # deg columns computed on PE (bank budget); rest on DVE
PREF = 6                 # stage1 adj group DMAs in flight
CLS_B_EVERY = 5          # stage2: every 5th chunk offloads PSUM reads to ACT + products to GP


def _chunks(total, step):
    return [(s, min(step, total - s)) for s in range(0, total, step)]


def build_stage1(nc_cores=NC):
    """Per-core: xnT shard [256, 1250] fp8 from DoubleRow-packed adjT + emb."""
    msh = MSH
    b = bacc.Bacc("TRN2", target_bir_lowering=False, debug=False, num_devices=nc_cores)
    # (p, g*GS*2*msh + s*2*msh + j*msh + m) = adjT[(g*GS+s)*256 + j*128 + p, m]
    adjx = b.dram_tensor("adjx", [128, KSUP * 2 * msh], F8, kind="ExternalInput")
    # (p, st*512 + j*256 + d) = emb[st*256 + j*128 + p, d]
    embx = b.dram_tensor("embx", [128, KSUP * 512], F8, kind="ExternalInput")
    embT = b.dram_tensor("embT", [D, msh], BF16, kind="ExternalInput")
    xnT = b.dram_tensor("xnT", [D, msh], F8, kind="ExternalOutput")

    ychunks = [(0, 512), (512, 512), (1024, msh - 1024)]
    dchunks = [(0, 512), (512, 512)]  # PE-deg columns
    dvw = msh - PEDEG                 # DVE-deg columns (226)

    with tile.TileContext(b) as tc:
        with (
            tc.tile_pool(name="const", bufs=1) as cpool,
            tc.tile_pool(name="stream", bufs=PREF) as spool,
            tc.tile_pool(name="work", bufs=2) as wpool,
            tc.tile_pool(name="acc", bufs=1, space="PSUM") as apool,
            tc.tile_pool(name="degp", bufs=1, space="PSUM") as dpool,
        ):
            embt = cpool.tile([128, KSUP * 512], F8, name="embt")
            EW = KSUP * 512 // 4

            def embt_dma(i):
                b.sync.dma_start(out=embt[:, i * EW : (i + 1) * EW],
                                 in_=embx.ap()[:, i * EW : (i + 1) * EW])

            grp = {}

            def grp_dma(g):
                t = spool.tile([128, GS * 2 * msh], F8, tag="adjg", name=f"adjg{g}")
                # two half-transfers to spread across more DMA queue rows
                h = GS * msh  # half the group columns
                base = g * 2 * GS * msh
                b.sync.dma_start(out=t[:, :h], in_=adjx.ap()[:, base : base + h])
                b.sync.dma_start(out=t[:, h:], in_=adjx.ap()[:, base + h : base + 2 * h])
                grp[g] = t

            embt_dma(0)
            for g in range(min(PREF, NG)):
                grp_dma(g)
            for i in range(1, 4):
                embt_dma(i)

            embT_sb = []
            for dh in range(2):
                t = cpool.tile([128, msh], BF16, name=f"embT{dh}")
                b.sync.dma_start(out=t[:], in_=embT.ap()[dh * 128 : (dh + 1) * 128, :])
                embT_sb.append(t)

            ones2 = cpool.tile([128, 32], F8, name="ones2")
            b.vector.memset(ones2[:], 1.0)
            ones_col = cpool.tile([128, 1], BF16, name="onescol")
            b.vector.memset(ones_col[:, :1], 1.0)
            bias_deg = cpool.tile([1, 1], F32, name="biasdeg")
            b.vector.memset(bias_deg[:1, :1], 1e-6)
            bias_ns = cpool.tile([1, 1], F32, name="biasns")
            b.vector.memset(bias_ns[:1, :1], 1e-16)

            ps_y = {
                (dh, c0): apool.tile(
                    [128, cw], F32, tag=f"py{dh}_{c0}", name=f"py{dh}_{c0}"
                )
                for dh in range(2)
                for (c0, cw) in ychunks
            }
            ps_d = {
                c0: dpool.tile([1, cw], F32, tag=f"pd{c0}", name=f"pd{c0}")
                for (c0, cw) in dchunks
            }
            # DVE-deg tail accumulators (2 chains so adds pipeline)
            dtail = [cpool.tile([128, 2 * dvw], BF16, name=f"dt{j}") for j in range(2)]

            for st in range(KSUP):
                g, s = divmod(st, GS)
                if g not in grp:
                    grp_dma(g)
                at = grp[g]
                base = s * 2 * msh
                at3 = at[:, base : base + 2 * msh].rearrange("p (j m) -> p j m", j=2)
                emb3 = embt[:, st * 512 : (st + 1) * 512].rearrange(
                    "p (j d) -> p j d", j=2
                )
                first, last = (st == 0), (st == KSUP - 1)
                for dh in range(2):
                    lhsT = emb3[:, :, dh * 128 : (dh + 1) * 128]
                    for (c0, cw) in ychunks:
                        b.tensor.matmul(
                            ps_y[(dh, c0)][:],
                            lhsT=lhsT,
                            rhs=at3[:, :, c0 : c0 + cw],
                            start=first,
                            stop=last,
                            perf_mode=DR,
                        )
                ones3 = ones2[:, :].rearrange("p (j o) -> p j o", j=2)
                for (c0, cw) in dchunks:
                    b.tensor.matmul(
                        ps_d[c0][:1, :],
                        lhsT=ones3[:, :, :1],
                        rhs=at3[:, :, c0 : c0 + cw],
                        start=first,
                        stop=last,
                        perf_mode=DR,
                    )
                # deg tail [PEDEG:msh] on DVE: fold both j-halves as columns
                tl = at[:, base + PEDEG : base + msh]
                th = at[:, base + msh + PEDEG : base + 2 * msh]
                j = st % 2
                if st < 2:
                    b.vector.tensor_copy(dtail[j][:, :dvw], tl)
                    b.vector.tensor_copy(dtail[j][:, dvw:], th)
                else:
                    b.vector.tensor_add(dtail[j][:, :dvw], dtail[j][:, :dvw], tl)
                    b.vector.tensor_add(dtail[j][:, dvw:], dtail[j][:, dvw:], th)
                if s == 0 and g + PREF < NG and (g + PREF) not in grp:
                    grp_dma(g + PREF)
                if last:
                    for gg in list(grp):
                        grp.pop(gg)

            # ---- deg tail: fold chains + halves, partition-reduce on gpsimd
            b.vector.tensor_add(dtail[0][:], dtail[0][:], dtail[1][:])
            dt_f = wpool.tile([128, dvw], BF16, tag="dtf")
            b.vector.tensor_add(dt_f[:], dtail[0][:, :dvw], dtail[0][:, dvw:])
            dt_r = wpool.tile([128, dvw], BF16, tag="dtr")
            b.gpsimd.partition_all_reduce(
                dt_r[:], dt_f[:], channels=128, reduce_op=bass.bass_isa.ReduceOp.add
            )

            # ---- epilogue: rinv = 1/(deg+1e-6) = exp(-ln(deg+1e-6)) on ACT
            rinv = wpool.tile([1, msh], F32, tag="rinv")
            for (c0, cw) in dchunks:
                b.scalar.activation(
                    rinv[:1, c0 : c0 + cw], ps_d[c0][:1, :], AF.Ln,
                    bias=bias_deg[:1, :1],
                )
            b.scalar.activation(
                rinv[:1, PEDEG:msh], dt_r[:1, :], AF.Ln, bias=bias_deg[:1, :1]
            )
            b.scalar.activation(rinv[:1, :], rinv[:1, :], AF.Exp, scale=-1.0)
            rinv_b = wpool.tile([128, msh], F32, tag="rinvb")
            b.gpsimd.partition_broadcast(rinv_b[:], rinv[:1, :], channels=128)

            xts = []
            sqs = []
            for dh in range(2):
                xt = wpool.tile([128, msh], F32, tag=f"xt{dh}")
                for (c0, cw) in ychunks:
                    b.vector.tensor_mul(
                        xt[:, c0 : c0 + cw], ps_y[(dh, c0)][:], rinv_b[:, c0 : c0 + cw]
                    )
                b.vector.tensor_add(xt[:], xt[:], embT_sb[dh][:])
                xts.append(xt)
                sq = wpool.tile([128, msh], BF16, tag=f"sq{dh}")
                b.scalar.activation(sq[:], xt[:], AF.Square)
                sqs.append(sq)
            # norm^2 column sums in 512-chunks reusing the freed deg banks
            rn = wpool.tile([1, msh], F32, tag="rn")
            for (c0, cw) in ychunks:
                ns_c = dpool.tile([1, 512], F32, tag=f"pd{c0 % 1024}", name=f"ns{c0}")
                for dh in range(2):
                    b.tensor.matmul(
                        ns_c[:1, :cw], lhsT=ones_col[:, :1],
                        rhs=sqs[dh][:, c0 : c0 + cw],
                        start=(dh == 0), stop=(dh == 1),
                    )
                b.scalar.activation(
                    rn[:1, c0 : c0 + cw], ns_c[:1, :cw], AF.Ln,
                    bias=bias_ns[:1, :1],
                )
            b.scalar.activation(rn[:1, :], rn[:1, :], AF.Exp, scale=-0.5)
            rn_b = wpool.tile([128, msh], F32, tag="rnb")
            b.gpsimd.partition_broadcast(rn_b[:], rn[:1, :], channels=128)
            for dh in range(2):
                xn = wpool.tile([128, msh], F8, tag=f"xn{dh}")
                b.vector.tensor_mul(xn[:], xts[dh][:], rn_b[:])
                b.sync.dma_start(
                    out=xnT.ap()[dh * 128 : (dh + 1) * 128, :], in_=xn[:]
                )
    b.compile()
    return b


def build_stage2(nc_cores=NC):
    """Per-core: w [512, 1].

    sumbuf_et = gather(tblW, src) then += gather(tblS slabs, dst) via SWDGE
    compute_op=add (descriptors must be <=2048B on the compute path, hence the
    5x2000 slab table).  Values {0,1,2}; common neighbor <=> sum >= 1.5.
    Per 1024-wide super-chunk (2 PSUM banks), three pipelines rotate:
      A (DVE):    t = (sum>=1.5)*cosL            [stt is_ge from PSUM]
      D (ACT+DVE): cnm=relu(sum-1); cosLs=copy;  t = cnm*cosLs  [DVE tt bf16 2x]
      G (ACT+GP):  same but GP does the multiply
    then always: parts[:,sc] = rowsum(t * cosR)  [DVE stt from PSUM + accum]
    """
    ql, etw, net = QL, ETW, NET
    r = 2 * ql
    SCW = 1024
    scs = _chunks(N, SCW)           # 9x1024 + 784
    nsc = len(scs)
    SLW = 2000                      # add-gather slab width (<=2048B)
    NSLAB = N // SLW

    b = bacc.Bacc(
        "TRN2",
        target_bir_lowering=False,
        debug=False,
        num_devices=nc_cores,
        dynamic_dma_scratch_size=65536,
    )
    # (p, j*N + c) = xn[j*128 + p, c]
    xns = b.dram_tensor("xns", [128, 2 * N], F8, kind="ExternalInput")
    tblW = b.dram_tensor("tblW", [r, N], F8, kind="ExternalInput")
    tblS = b.dram_tensor("tblS", [NSLAB, r, SLW], F8, kind="ExternalInput")
    idxs = b.dram_tensor("idxs", [ql, 1], I32, kind="ExternalInput")
    idxd = b.dram_tensor("idxd", [ql, 1], I32, kind="ExternalInput")
    # (p, j*ql + e) = xn[j*128 + p, endpoint_e]
    ut = b.dram_tensor("ut", [128, 2 * ql], F8, kind="ExternalInput")
    vt = b.dram_tensor("vt", [128, 2 * ql], F8, kind="ExternalInput")
    w = b.dram_tensor("w", [ql, 1], F32, kind="ExternalOutput")

    def pattern(sci, et):
        m = sci % 6
        if m == 0:
            return "A"
        if m in (4, 5) and et >= 2:
            return "G"
        return "D"

    with tile.TileContext(b) as tc:
        with (
            tc.tile_pool(name="const", bufs=1) as cpool,
            tc.tile_pool(name="gather", bufs=4) as gpool,
            tc.tile_pool(name="mid", bufs=4) as dpool,
            tc.tile_pool(name="small", bufs=2) as wpool,
            tc.tile_pool(name="cos", bufs=2, space="PSUM") as ppool,
        ):
            ix_s, ix_d = [], []
            for et in range(net):
                ts_ = cpool.tile([etw, 1], I32, tag=f"ixs{et}")
                b.sync.dma_start(out=ts_[:], in_=idxs.ap()[et * etw : (et + 1) * etw, :1])
                ix_s.append(ts_)
                td_ = cpool.tile([etw, 1], I32, tag=f"ixd{et}")
                b.sync.dma_start(out=td_[:], in_=idxd.ap()[et * etw : (et + 1) * etw, :1])
                ix_d.append(td_)

            # resident xn + endpoint tiles
            xnt = cpool.tile([128, 2 * N], F8, name="xnt")
            for i in range(4):
                cw = 2 * N // 4
                b.sync.dma_start(out=xnt[:, i * cw : (i + 1) * cw],
                                 in_=xns.ap()[:, i * cw : (i + 1) * cw])
            biasm1 = cpool.tile([128, 1], F32, name="biasm1")
            b.vector.memset(biasm1[:, :1], -1.0)
            utt = cpool.tile([128, 2 * ql], F8, name="utt")
            b.sync.dma_start(out=utt[:], in_=ut.ap()[:, :])
            vtt = cpool.tile([128, 2 * ql], F8, name="vtt")
            b.sync.dma_start(out=vtt[:], in_=vt.ap()[:, :])

            # gathers first: sumbuf = adj[src] (wide) += adj[dst] (slab chunks)
            pend = {}
            for et in range(net):
                sb_ = gpool.tile([etw, N], F8, tag="sb", bufs=4, name=f"sb{et}")
                b.gpsimd.indirect_dma_start(
                    out=sb_[:], out_offset=None, in_=tblW.ap(),
                    in_offset=bass.IndirectOffsetOnAxis(ap=ix_s[et][:, :1], axis=0),
                )
                for k in range(NSLAB):
                    b.gpsimd.indirect_dma_start(
                        out=sb_[:, k * SLW : (k + 1) * SLW], out_offset=None,
                        in_=tblS.ap()[0],
                        in_offset=bass.IndirectOffsetOnAxis(ap=ix_d[et][:, :1], axis=0),
                        compute_op=OP.add, element_offset=k * r * SLW,
                    )
                pend[et] = sb_

            xn3 = xnt[:, :].rearrange("p (j c) -> p j c", j=2)
            ut3 = utt[:, :].rearrange("p (j e) -> p j e", j=2)
            vt3 = vtt[:, :].rearrange("p (j e) -> p j e", j=2)

            # alpha-prefill: tile 0's first ALPHA sc compute P = cosL*cosR
            # during the gather-descriptor window (no mask needed yet)
            ALPHA = 8
            ptiles = [cpool.tile([etw, SCW], BF16, name=f"pp{i}") for i in range(ALPHA)]

            sci = 0
            for et in range(net):
                esl = slice(et * etw, (et + 1) * etw)
                sb_ = pend.pop(et)
                parts = wpool.tile([etw, nsc], F32, tag="parts")
                if et == 0:
                    for ci in range(ALPHA):
                        c0, cw = scs[ci]
                        cosL = ppool.tile([etw, SCW], F32, tag="cosL")
                        cosR = ppool.tile([etw, SCW], F32, tag="cosR")
                        for h0, hw in _chunks(cw, 512):
                            b.tensor.matmul(
                                cosL[:, h0 : h0 + hw], lhsT=ut3[:, :, esl],
                                rhs=xn3[:, :, c0 + h0 : c0 + h0 + hw],
                                start=True, stop=True, perf_mode=DR,
                            )
                        for h0, hw in _chunks(cw, 512):
                            b.tensor.matmul(
                                cosR[:, h0 : h0 + hw], lhsT=vt3[:, :, esl],
                                rhs=xn3[:, :, c0 + h0 : c0 + h0 + hw],
                                start=True, stop=True, perf_mode=DR,
                            )
                        cosRs = dpool.tile([etw, SCW], BF16, tag="cosRs")
                        b.scalar.activation(cosRs[:, :cw], cosR[:, :cw], AF.Copy)
                        b.vector.tensor_mul(ptiles[ci][:, :cw], cosL[:, :cw],
                                            cosRs[:, :cw])
                    for ci in range(ALPHA):
                        c0, cw = scs[ci]
                        scr = dpool.tile([etw, SCW], BF16, tag="scr")
                        b.vector.scalar_tensor_tensor(
                            out=scr[:, :cw], in0=sb_[:, c0 : c0 + cw],
                            scalar=1.5, in1=ptiles[ci][:, :cw],
                            op0=OP.is_ge, op1=OP.mult,
                            accum_out=parts[:, ci : ci + 1],
                        )
                        sci += 1
                for ci, (c0, cw) in enumerate(scs):
                    if et == 0 and ci < ALPHA:
                        continue
                    cosL = ppool.tile([etw, SCW], F32, tag="cosL")
                    cosR = ppool.tile([etw, SCW], F32, tag="cosR")
                    for h0, hw in _chunks(cw, 512):
                        b.tensor.matmul(
                            cosL[:, h0 : h0 + hw], lhsT=ut3[:, :, esl],
                            rhs=xn3[:, :, c0 + h0 : c0 + h0 + hw],
                            start=True, stop=True, perf_mode=DR,
                        )
                    for h0, hw in _chunks(cw, 512):
                        b.tensor.matmul(
                            cosR[:, h0 : h0 + hw], lhsT=vt3[:, :, esl],
                            rhs=xn3[:, :, c0 + h0 : c0 + h0 + hw],
                            start=True, stop=True, perf_mode=DR,
                        )
                    t_ = dpool.tile([etw, SCW], BF16, tag="t_")
                    pat = pattern(sci, et)
                    if pat == "A":
                        b.vector.scalar_tensor_tensor(
                            out=t_[:, :cw], in0=sb_[:, c0 : c0 + cw],
                            scalar=1.5, in1=cosL[:, :cw],
                            op0=OP.is_ge, op1=OP.mult,
                        )
                    else:
                        cosLs = dpool.tile([etw, SCW], BF16, tag="cosLs")
                        b.scalar.activation(cosLs[:, :cw], cosL[:, :cw], AF.Copy)
                        cnm = dpool.tile([etw, SCW], BF16, tag="cnm")
                        b.scalar.activation(
                            cnm[:, :cw], sb_[:, c0 : c0 + cw], AF.Relu,
                            bias=biasm1[:, :1],
                        )
                        if pat == "G":
                            b.gpsimd.tensor_mul(t_[:, :cw], cnm[:, :cw], cosLs[:, :cw])
                        else:
                            b.vector.tensor_mul(t_[:, :cw], cnm[:, :cw], cosLs[:, :cw])
                    scr = dpool.tile([etw, SCW], BF16, tag="scr")
                    b.vector.scalar_tensor_tensor(
                        out=scr[:, :cw],
                        in0=t_[:, :cw],
                        scalar=1.0,
                        in1=cosR[:, :cw],
                        op0=OP.mult,
                        op1=OP.mult,
                        accum_out=parts[:, ci : ci + 1],
                    )
                    sci += 1
                wacc = wpool.tile([etw, 1], F32, tag="wacc")
                b.vector.reduce_sum(wacc[:, :1], parts[:], axis=mybir.AxisListType.X)
                sg = wpool.tile([etw, 1], F32, tag="sg")
                b.scalar.activation(sg[:, :1], wacc[:, :1], AF.Sigmoid)
                b.sync.dma_start(out=w.ap()[et * etw : (et + 1) * etw, :1], in_=sg[:, :1])
    b.compile()
    return b


def make_stage1_inputs(emb, adj, nc_cores=NC):
    msh = MSH
    e_pad = np.zeros((KP2, D), np.float32)
    e_pad[:N] = emb
    # (p, st*512 + j*256 + d) = emb[st*256 + j*128 + p, d]
    embx = np.ascontiguousarray(
        e_pad.reshape(KSUP, 2, 128, D).transpose(2, 0, 1, 3).reshape(128, KSUP * 512)
    ).astype(NP_F8)
    ins = []
    for k in range(nc_cores):
        sh = adj[k * msh : (k + 1) * msh, :]  # [msh, N] rows of this core
        adjT = np.zeros((KP2, msh), np.float32)
        adjT[:N] = sh.T
        # (p, st*2*msh + j*msh + m) = adjT[st*256 + j*128 + p, m]
        adjx = np.ascontiguousarray(
            adjT.reshape(KSUP, 2, 128, msh).transpose(2, 0, 1, 3).reshape(128, KSUP * 2 * msh)
        ).astype(NP_F8)
        embT = np.ascontiguousarray(emb[k * msh : (k + 1) * msh, :].T).astype(NP_BF16)
        ins.append({"adjx": adjx, "embx": embx, "embT": embT})
    return ins


def make_stage2_inputs(adj, xnT, src, dst_, nc_cores=NC):
    ql = QL
    # xns: (p, j*N + c) = xnT[j*128 + p, c]
    xns = np.ascontiguousarray(xnT.reshape(2, 128, N).transpose(1, 0, 2).reshape(128, 2 * N))
    xnT_f32 = xnT.astype(np.float32)
    ins = []
    for k in range(nc_cores):
        s_k = src[k * ql : (k + 1) * ql]
        d_k = dst_[k * ql : (k + 1) * ql]
        uniq = np.unique(np.concatenate([s_k, d_k]))
        tblW = np.zeros((2 * ql, N), NP_F8)
        tblW[: len(uniq)] = adj[uniq].astype(NP_F8)
        SLW = 2000
        # [NSLAB, r, SLW] column-slab layout for the accumulating gathers
        tblS = np.ascontiguousarray(
            tblW.reshape(2 * ql, N // SLW, SLW).transpose(1, 0, 2)
        )

        def pack_cols(cols):
            # [256, ql] -> (p, j*ql + e)
            g = xnT_f32[:, cols]
            return np.ascontiguousarray(
                g.reshape(2, 128, ql).transpose(1, 0, 2).reshape(128, 2 * ql)
            ).astype(NP_F8)

        ins.append(
            {
                "xns": xns,
                "tblW": tblW,
                "tblS": tblS,
                "idxs": np.searchsorted(uniq, s_k).astype(np.int32)[:, None],
                "idxd": np.searchsorted(uniq, d_k).astype(np.int32)[:, None],
                "ut": pack_cols(s_k),
                "vt": pack_cols(d_k),
            }
        )
    return ins


_progs = {}
LAST_RESULTS = []  # BassKernelResults of the most recent kernel() call (for profiling)


def _get(name, builder):
    if name not in _progs:
        _progs[name] = builder()
    return _progs[name]


def kernel(emb_weight, adj, edges):
    emb = np.asarray(emb_weight, dtype=np.float32)
    adj = np.asarray(adj, dtype=np.float32)
    edges = np.asarray(edges)
    src = edges[0].astype(np.int64)
    dst_ = edges[1].astype(np.int64)

    s1 = _get("s1", build_stage1)
    s2 = _get("s2", build_stage2)

    in1 = make_stage1_inputs(emb, adj)
    r1 = bass_utils.run_bass_kernel_spmd(s1, in1, core_ids=list(range(NC)))
    xnT = np.concatenate(
        [np.asarray(r1.results[k]["xnT"]) for k in range(NC)], axis=1
    )

    in2 = make_stage2_inputs(adj, xnT, src, dst_)
    r2 = bass_utils.run_bass_kernel_spmd(s2, in2, core_ids=list(range(NC)))
    w = np.concatenate([r2.results[k]["w"][:, 0] for k in range(NC)])

    LAST_RESULTS.clear()
    LAST_RESULTS.extend([r1, r2])
    return w.astype(np.float32)



# revision 12
# speedup vs baseline: 1.0224x; 1.0222x over previous
"""CommonNeighborsPredictor kernel for 8 Trainium2 NeuronCores.

Math (see reference):
    deg = adj.sum(-1) + 1e-6
    x   = emb + (adj @ emb) / deg[:, None]
    xn  = x / max(||x||_2, 1e-8)                            # row-normalize
    w_e = sum_c adj[src_e, c] * adj[dst_e, c] * (xn[src_e]@xn[c]) * (xn[dst_e]@xn[c])
    out = sigmoid(w)

Distribution (2 SPMD launches, no collectives):
  Stage 1: shard nodes (rows of adj) 8 ways; each core computes xn^T for its
    1250 nodes.  The adjacency is fed as fp8_e4m3 (0/1 values are exact) in a
    DoubleRow-interleaved layout so the PE contracts K=256 per matmul at 2x
    fp8 rate.  Degrees ride the PE as an extra M=1 ones-matmul for columns
    [0:1024] (PSUM bank budget) and a small DVE+gpsimd fold for [1024:1250].
    The normalization epilogue uses activation-engine Dsqrt for 1/deg and
    1/||x|| (Square(Dsqrt(x/4)) == 1/x) and gpsimd partition_broadcast
    instead of PE broadcast matmuls.
  Stage 2: shard query edges 8 ways (512 each).  Adjacency rows for the two
    endpoints are indirect-DMA-gathered in fp8 from a per-core dedup'd table;
    gpsimd computes the common-neighbor mask cn = aS*aD (bf16 out); the two
    cos matrices come from DoubleRow fp8 matmuls against resident interleaved
    xn; the scalar engine copies cosR out of PSUM to bf16 so the DVE mask
    multiply runs at 2x, and a fused scalar_tensor_tensor does the final
    product + row-reduction.  Sigmoid on the scalar engine.

dtypes: adjacency and xn are fp8_e4m3 (adjacency exact; xn rounding gives
~1e-3 max output error vs the fp32 reference).  PSUM accumulation and the
normalization epilogue are fp32; masks/products bf16.
"""

import numpy as np

import concourse.bass as bass
import concourse.bacc as bacc
import concourse.mybir as mybir
import concourse.tile as tile
from concourse import bass_utils

F32 = mybir.dt.float32
BF16 = mybir.dt.bfloat16
F8 = mybir.dt.float8e4
I32 = mybir.dt.int32
AF = mybir.ActivationFunctionType
OP = mybir.AluOpType
DR = mybir.MatmulPerfMode.DoubleRow
NP_BF16 = mybir.dt.np(BF16)
NP_F8 = mybir.dt.np(F8)

N, D, Q, NC = 10000, 256, 4096, 8
MSH = N // NC            # 1250 nodes per core (stage 1)
KSUP = 40                # k super-tiles of 256 rows (10240 padded)
KP2 = KSUP * 256
GS = 4                   # super-tiles per DMA group (1.28 MB each)
NG = KSUP // GS
QL = Q // NC             # 512 edges per core (stage 2)
ETW = 128                # edges per tile
NET = QL // ETW
PEDEG = 1024             # deg columns computed on PE (bank budget); rest on DVE
PREF = 6                 # stage1 adj group DMAs in flight
CLS_B_EVERY = 5          # stage2: every 5th chunk offloads PSUM reads to ACT + products to GP


def _chunks(total, step):
    return [(s, min(step, total - s)) for s in range(0, total, step)]


def build_stage1(nc_cores=NC):
    """Per-core: xnT shard [256, 1250] fp8 from DoubleRow-packed adjT + emb."""
    msh = MSH
    b = bacc.Bacc("TRN2", target_bir_lowering=False, debug=False, num_devices=nc_cores)
    # (p, g*GS*2*msh + s*2*msh + j*msh + m) = adjT[(g*GS+s)*256 + j*128 + p, m]
    adjx = b.dram_tensor("adjx", [128, KSUP * 2 * msh], F8, kind="ExternalInput")
    # (p, st*512 + j*256 + d) = emb[st*256 + j*128 + p, d]
    embx = b.dram_tensor("embx", [128, KSUP * 512], F8, kind="ExternalInput")
    embT = b.dram_tensor("embT", [D, msh], BF16, kind="ExternalInput")
    xnT = b.dram_tensor("xnT", [D, msh], F8, kind="ExternalOutput")

    ychunks = [(0, 512), (512, 512), (1024, msh - 1024)]
    dchunks = [(0, 512), (512, 512)]  # PE-deg columns
    dvw = msh - PEDEG                 # DVE-deg columns (226)

    with tile.TileContext(b) as tc:
        with (
            tc.tile_pool(name="const", bufs=1) as cpool,
            tc.tile_pool(name="stream", bufs=PREF) as spool,
            tc.tile_pool(name="work", bufs=2) as wpool,
            tc.tile_pool(name="acc", bufs=1, space="PSUM") as apool,
            tc.tile_pool(name="degp", bufs=1, space="PSUM") as dpool,
        ):
            embt = cpool.tile([128, KSUP * 512], F8, name="embt")
            EW = KSUP * 512 // 4

            def embt_dma(i):
                b.sync.dma_start(out=embt[:, i * EW : (i + 1) * EW],
                                 in_=embx.ap()[:, i * EW : (i + 1) * EW])

            grp = {}

            def grp_dma(g):
                t = spool.tile([128, GS * 2 * msh], F8, tag="adjg", name=f"adjg{g}")
                # two half-transfers to spread across more DMA queue rows
                h = GS * msh  # half the group columns
                base = g * 2 * GS * msh
                b.sync.dma_start(out=t[:, :h], in_=adjx.ap()[:, base : base + h])
                b.sync.dma_start(out=t[:, h:], in_=adjx.ap()[:, base + h : base + 2 * h])
                grp[g] = t

            embt_dma(0)
            for g in range(min(PREF, NG)):
                grp_dma(g)
            for i in range(1, 4):
                embt_dma(i)

            embT_sb = []
            for dh in range(2):
                t = cpool.tile([128, msh], BF16, name=f"embT{dh}")
                b.sync.dma_start(out=t[:], in_=embT.ap()[dh * 128 : (dh + 1) * 128, :])
                embT_sb.append(t)

            ones2 = cpool.tile([128, 32], F8, name="ones2")
            b.vector.memset(ones2[:], 1.0)
            ones_col = cpool.tile([128, 1], BF16, name="onescol")
            b.vector.memset(ones_col[:, :1], 1.0)
            bias_deg = cpool.tile([1, 1], F32, name="biasdeg")
            b.vector.memset(bias_deg[:1, :1], 1e-6)
            bias_ns = cpool.tile([1, 1], F32, name="biasns")
            b.vector.memset(bias_ns[:1, :1], 1e-16)

            ps_y = {
                (dh, c0): apool.tile(
                    [128, cw], F32, tag=f"py{dh}_{c0}", name=f"py{dh}_{c0}"
                )
                for dh in range(2)
                for (c0, cw) in ychunks
            }
            ps_d = {
                c0: dpool.tile([1, cw], F32, tag=f"pd{c0}", name=f"pd{c0}")
                for (c0, cw) in dchunks
            }
            # DVE-deg tail accumulators (2 chains so adds pipeline)
            dtail = [cpool.tile([128, 2 * dvw], BF16, name=f"dt{j}") for j in range(2)]

            for st in range(KSUP):
                g, s = divmod(st, GS)
                if g not in grp:
                    grp_dma(g)
                at = grp[g]
                base = s * 2 * msh
                at3 = at[:, base : base + 2 * msh].rearrange("p (j m) -> p j m", j=2)
                emb3 = embt[:, st * 512 : (st + 1) * 512].rearrange(
                    "p (j d) -> p j d", j=2
                )
                first, last = (st == 0), (st == KSUP - 1)
                for dh in range(2):
                    lhsT = emb3[:, :, dh * 128 : (dh + 1) * 128]
                    for (c0, cw) in ychunks:
                        b.tensor.matmul(
                            ps_y[(dh, c0)][:],
                            lhsT=lhsT,
                            rhs=at3[:, :, c0 : c0 + cw],
                            start=first,
                            stop=last,
                            perf_mode=DR,
                        )
                ones3 = ones2[:, :].rearrange("p (j o) -> p j o", j=2)
                for (c0, cw) in dchunks:
                    b.tensor.matmul(
                        ps_d[c0][:1, :],
                        lhsT=ones3[:, :, :1],
                        rhs=at3[:, :, c0 : c0 + cw],
                        start=first,
                        stop=last,
                        perf_mode=DR,
                    )
                # deg tail [PEDEG:msh] on DVE: fold both j-halves as columns
                tl = at[:, base + PEDEG : base + msh]
                th = at[:, base + msh + PEDEG : base + 2 * msh]
                j = st % 2
                if st < 2:
                    b.vector.tensor_copy(dtail[j][:, :dvw], tl)
                    b.vector.tensor_copy(dtail[j][:, dvw:], th)
                else:
                    b.vector.tensor_add(dtail[j][:, :dvw], dtail[j][:, :dvw], tl)
                    b.vector.tensor_add(dtail[j][:, dvw:], dtail[j][:, dvw:], th)
                if s == 0 and g + PREF < NG and (g + PREF) not in grp:
                    grp_dma(g + PREF)
                if last:
                    for gg in list(grp):
                        grp.pop(gg)

            # ---- deg tail: fold chains + halves, partition-reduce on gpsimd
            b.vector.tensor_add(dtail[0][:], dtail[0][:], dtail[1][:])
            dt_f = wpool.tile([128, dvw], BF16, tag="dtf")
            b.vector.tensor_add(dt_f[:], dtail[0][:, :dvw], dtail[0][:, dvw:])
            dt_r = wpool.tile([128, dvw], BF16, tag="dtr")
            b.gpsimd.partition_all_reduce(
                dt_r[:], dt_f[:], channels=128, reduce_op=bass.bass_isa.ReduceOp.add
            )

            # ---- epilogue: rinv = 1/(deg+1e-6) = exp(-ln(deg+1e-6)) on ACT
            rinv = wpool.tile([1, msh], F32, tag="rinv")
            for (c0, cw) in dchunks:
                b.scalar.activation(
                    rinv[:1, c0 : c0 + cw], ps_d[c0][:1, :], AF.Ln,
                    bias=bias_deg[:1, :1],
                )
            b.scalar.activation(
                rinv[:1, PEDEG:msh], dt_r[:1, :], AF.Ln, bias=bias_deg[:1, :1]
            )
            b.scalar.activation(rinv[:1, :], rinv[:1, :], AF.Exp, scale=-1.0)
            rinv_b = wpool.tile([128, msh], F32, tag="rinvb")
            b.gpsimd.partition_broadcast(rinv_b[:], rinv[:1, :], channels=128)

            xts = []
            sqs = []
            for dh in range(2):
                xt = wpool.tile([128, msh], F32, tag=f"xt{dh}")
                for (c0, cw) in ychunks:
                    b.vector.tensor_mul(
                        xt[:, c0 : c0 + cw], ps_y[(dh, c0)][:], rinv_b[:, c0 : c0 + cw]
                    )
                b.vector.tensor_add(xt[:], xt[:], embT_sb[dh][:])
                xts.append(xt)
                sq = wpool.tile([128, msh], BF16, tag=f"sq{dh}")
                b.scalar.activation(sq[:], xt[:], AF.Square)
                sqs.append(sq)
            # norm^2 column sums in 512-chunks reusing the freed deg banks
            rn = wpool.tile([1, msh], F32, tag="rn")
            for (c0, cw) in ychunks:
                ns_c = dpool.tile([1, 512], F32, tag=f"pd{c0 % 1024}", name=f"ns{c0}")
                for dh in range(2):
                    b.tensor.matmul(
                        ns_c[:1, :cw], lhsT=ones_col[:, :1],
                        rhs=sqs[dh][:, c0 : c0 + cw],
                        start=(dh == 0), stop=(dh == 1),
                    )
                b.scalar.activation(
                    rn[:1, c0 : c0 + cw], ns_c[:1, :cw], AF.Ln,
                    bias=bias_ns[:1, :1],
                )
            b.scalar.activation(rn[:1, :], rn[:1, :], AF.Exp, scale=-0.5)
            rn_b = wpool.tile([128, msh], F32, tag="rnb")
            b.gpsimd.partition_broadcast(rn_b[:], rn[:1, :], channels=128)
            for dh in range(2):
                xn = wpool.tile([128, msh], F8, tag=f"xn{dh}")
                b.vector.tensor_mul(xn[:], xts[dh][:], rn_b[:])
                b.sync.dma_start(
                    out=xnT.ap()[dh * 128 : (dh + 1) * 128, :], in_=xn[:]
                )
    b.compile()
    return b


def build_stage2(nc_cores=NC):
    """Per-core: w [512, 1].

    sumbuf_et = gather(tblW, src) then += gather(tblS slabs, dst) via SWDGE
    compute_op=add (descriptors must be <=2048B on the compute path, hence the
    5x2000 slab table).  Values {0,1,2}; common neighbor <=> sum >= 1.5.
    Per 1024-wide super-chunk (2 PSUM banks), three pipelines rotate:
      A (DVE):    t = (sum>=1.5)*cosL            [stt is_ge from PSUM]
      D (ACT+DVE): cnm=relu(sum-1); cosLs=copy;  t = cnm*cosLs  [DVE tt bf16 2x]
      G (ACT+GP):  same but GP does the multiply
    then always: parts[:,sc] = rowsum(t * cosR)  [DVE stt from PSUM + accum]
    """
    ql, etw, net = QL, ETW, NET
    r = 2 * ql
    SCW = 1024
    scs = _chunks(N, SCW)           # 9x1024 + 784
    nsc = len(scs)
    SLW = 2000                      # add-gather slab width (<=2048B)
    NSLAB = N // SLW

    b = bacc.Bacc(
        "TRN2",
        target_bir_lowering=False,
        debug=False,
        num_devices=nc_cores,
        dynamic_dma_scratch_size=65536,
    )
    # (p, j*N + c) = xn[j*128 + p, c]
    xns = b.dram_tensor("xns", [128, 2 * N], F8, kind="ExternalInput")
    tblW = b.dram_tensor("tblW", [r, N], F8, kind="ExternalInput")
    tblS = b.dram_tensor("tblS", [NSLAB, r, SLW], F8, kind="ExternalInput")
    idxs = b.dram_tensor("idxs", [ql, 1], I32, kind="ExternalInput")
    idxd = b.dram_tensor("idxd", [ql, 1], I32, kind="ExternalInput")
    # (p, j*ql + e) = xn[j*128 + p, endpoint_e]
    ut = b.dram_tensor("ut", [128, 2 * ql], F8, kind="ExternalInput")
    vt = b.dram_tensor("vt", [128, 2 * ql], F8, kind="ExternalInput")
    w = b.dram_tensor("w", [ql, 1], F32, kind="ExternalOutput")

    def pattern(sci, et):
        m = sci % 6
        if m == 0:
            return "A"
        if m in (4, 5) and et >= 2:
            return "G"
        return "D"

    with tile.TileContext(b) as tc:
        with (
            tc.tile_pool(name="const", bufs=1) as cpool,
            tc.tile_pool(name="gather", bufs=4) as gpool,
            tc.tile_pool(name="mid", bufs=4) as dpool,
            tc.tile_pool(name="small", bufs=2) as wpool,
            tc.tile_pool(name="cos", bufs=2, space="PSUM") as ppool,
        ):
            ix_s, ix_d = [], []
            for et in range(net):
                ts_ = cpool.tile([etw, 1], I32, tag=f"ixs{et}")
                b.sync.dma_start(out=ts_[:], in_=idxs.ap()[et * etw : (et + 1) * etw, :1])
                ix_s.append(ts_)
                td_ = cpool.tile([etw, 1], I32, tag=f"ixd{et}")
                b.sync.dma_start(out=td_[:], in_=idxd.ap()[et * etw : (et + 1) * etw, :1])
                ix_d.append(td_)

            # resident xn + endpoint tiles
            xnt = cpool.tile([128, 2 * N], F8, name="xnt")
            for i in range(4):
                cw = 2 * N // 4
                b.sync.dma_start(out=xnt[:, i * cw : (i + 1) * cw],
                                 in_=xns.ap()[:, i * cw : (i + 1) * cw])
            biasm1 = cpool.tile([128, 1], F32, name="biasm1")
            b.vector.memset(biasm1[:, :1], -1.0)
            utt = cpool.tile([128, 2 * ql], F8, name="utt")
            b.sync.dma_start(out=utt[:], in_=ut.ap()[:, :])
            vtt = cpool.tile([128, 2 * ql], F8, name="vtt")
            b.sync.dma_start(out=vtt[:], in_=vt.ap()[:, :])

            # delay gathers until xnt has landed (the 25MB of gather data
            # otherwise starves the xnt DMA and the PE idles ~30us):
            # derive a zero from xnt and add it to the src index tiles.
            zfx = cpool.tile([etw, 4], BF16, name="zfx")
            b.vector.tensor_scalar(out=zfx[:], in0=xnt[:, ::5000], scalar1=-300.0,
                                   scalar2=None, op0=OP.is_lt)
            zix = cpool.tile([etw, 1], I32, name="zix")
            b.vector.tensor_copy(zix[:, :1], zfx[:, :1])

            # gathers: sumbuf = adj[src] (wide) += adj[dst] (slab chunks)
            pend = {}
            for et in range(net):
                ixg = cpool.tile([etw, 1], I32, tag=f"ixg{et}")
                b.vector.tensor_add(ixg[:, :1], ix_s[et][:, :1], zix[:, :1])
                sb_ = gpool.tile([etw, N], F8, tag="sb", bufs=4, name=f"sb{et}")
                b.gpsimd.indirect_dma_start(
                    out=sb_[:], out_offset=None, in_=tblW.ap(),
                    in_offset=bass.IndirectOffsetOnAxis(ap=ixg[:, :1], axis=0),
                )
                for k in range(NSLAB):
                    b.gpsimd.indirect_dma_start(
                        out=sb_[:, k * SLW : (k + 1) * SLW], out_offset=None,
                        in_=tblS.ap()[0],
                        in_offset=bass.IndirectOffsetOnAxis(ap=ix_d[et][:, :1], axis=0),
                        compute_op=OP.add, element_offset=k * r * SLW,
                    )
                pend[et] = sb_

            xn3 = xnt[:, :].rearrange("p (j c) -> p j c", j=2)
            ut3 = utt[:, :].rearrange("p (j e) -> p j e", j=2)
            vt3 = vtt[:, :].rearrange("p (j e) -> p j e", j=2)

            # alpha-prefill: tile 0's first ALPHA sc compute P = cosL*cosR
            # during the gather-descriptor window (no mask needed yet)
            ALPHA = 8
            ptiles = [cpool.tile([etw, SCW], BF16, name=f"pp{i}") for i in range(ALPHA)]

            sci = 0
            for et in range(net):
                esl = slice(et * etw, (et + 1) * etw)
                sb_ = pend.pop(et)
                parts = wpool.tile([etw, nsc], F32, tag="parts")
                if et == 0:
                    for ci in range(ALPHA):
                        c0, cw = scs[ci]
                        cosL = ppool.tile([etw, SCW], F32, tag="cosL")
                        cosR = ppool.tile([etw, SCW], F32, tag="cosR")
                        for h0, hw in _chunks(cw, 512):
                            b.tensor.matmul(
                                cosL[:, h0 : h0 + hw], lhsT=ut3[:, :, esl],
                                rhs=xn3[:, :, c0 + h0 : c0 + h0 + hw],
                                start=True, stop=True, perf_mode=DR,
                            )
                        for h0, hw in _chunks(cw, 512):
                            b.tensor.matmul(
                                cosR[:, h0 : h0 + hw], lhsT=vt3[:, :, esl],
                                rhs=xn3[:, :, c0 + h0 : c0 + h0 + hw],
                                start=True, stop=True, perf_mode=DR,
                            )
                        cosRs = dpool.tile([etw, SCW], BF16, tag="cosRs")
                        b.scalar.activation(cosRs[:, :cw], cosR[:, :cw], AF.Copy)
                        b.vector.tensor_mul(ptiles[ci][:, :cw], cosL[:, :cw],
                                            cosRs[:, :cw])
                    for ci in range(ALPHA):
                        c0, cw = scs[ci]
                        scr = dpool.tile([etw, SCW], BF16, tag="scr")
                        b.vector.scalar_tensor_tensor(
                            out=scr[:, :cw], in0=sb_[:, c0 : c0 + cw],
                            scalar=1.5, in1=ptiles[ci][:, :cw],
                            op0=OP.is_ge, op1=OP.mult,
                            accum_out=parts[:, ci : ci + 1],
                        )
                        sci += 1
                for ci, (c0, cw) in enumerate(scs):
                    if et == 0 and ci < ALPHA:
                        continue
                    cosL = ppool.tile([etw, SCW], F32, tag="cosL")
                    cosR = ppool.tile([etw, SCW], F32, tag="cosR")
                    for h0, hw in _chunks(cw, 512):
                        b.tensor.matmul(
                            cosL[:, h0 : h0 + hw], lhsT=ut3[:, :, esl],
                            rhs=xn3[:, :, c0 + h0 : c0 + h0 + hw],
                            start=True, stop=True, perf_mode=DR,
                        )
                    for h0, hw in _chunks(cw, 512):
                        b.tensor.matmul(
                            cosR[:, h0 : h0 + hw], lhsT=vt3[:, :, esl],
                            rhs=xn3[:, :, c0 + h0 : c0 + h0 + hw],
                            start=True, stop=True, perf_mode=DR,
                        )
                    t_ = dpool.tile([etw, SCW], BF16, tag="t_")
                    pat = pattern(sci, et)
                    if pat == "A":
                        b.vector.scalar_tensor_tensor(
                            out=t_[:, :cw], in0=sb_[:, c0 : c0 + cw],
                            scalar=1.5, in1=cosL[:, :cw],
                            op0=OP.is_ge, op1=OP.mult,
                        )
                    else:
                        cosLs = dpool.tile([etw, SCW], BF16, tag="cosLs")
                        b.scalar.activation(cosLs[:, :cw], cosL[:, :cw], AF.Copy)
                        cnm = dpool.tile([etw, SCW], BF16, tag="cnm")
                        b.scalar.activation(
                            cnm[:, :cw], sb_[:, c0 : c0 + cw], AF.Relu,
                            bias=biasm1[:, :1],
                        )
                        if pat == "G":
                            b.gpsimd.tensor_mul(t_[:, :cw], cnm[:, :cw], cosLs[:, :cw])
                        else:
                            b.vector.tensor_mul(t_[:, :cw], cnm[:, :cw], cosLs[:, :cw])
                    scr = dpool.tile([etw, SCW], BF16, tag="scr")
                    b.vector.scalar_tensor_tensor(
                        out=scr[:, :cw],
                        in0=t_[:, :cw],
                        scalar=1.0,
                        in1=cosR[:, :cw],
                        op0=OP.mult,
                        op1=OP.mult,
                        accum_out=parts[:, ci : ci + 1],
                    )
                    sci += 1
                wacc = wpool.tile([etw, 1], F32, tag="wacc")
                b.vector.reduce_sum(wacc[:, :1], parts[:], axis=mybir.AxisListType.X)
                sg = wpool.tile([etw, 1], F32, tag="sg")
                b.scalar.activation(sg[:, :1], wacc[:, :1], AF.Sigmoid)
                b.sync.dma_start(out=w.ap()[et * etw : (et + 1) * etw, :1], in_=sg[:, :1])
    b.compile()
    return b


def make_stage1_inputs(emb, adj, nc_cores=NC):
    msh = MSH
    e_pad = np.zeros((KP2, D), np.float32)
    e_pad[:N] = emb
    # (p, st*512 + j*256 + d) = emb[st*256 + j*128 + p, d]
    embx = np.ascontiguousarray(
        e_pad.reshape(KSUP, 2, 128, D).transpose(2, 0, 1, 3).reshape(128, KSUP * 512)
    ).astype(NP_F8)
    ins = []
    for k in range(nc_cores):
        sh = adj[k * msh : (k + 1) * msh, :]  # [msh, N] rows of this core
        adjT = np.zeros((KP2, msh), np.float32)
        adjT[:N] = sh.T
        # (p, st*2*msh + j*msh + m) = adjT[st*256 + j*128 + p, m]
        adjx = np.ascontiguousarray(
            adjT.reshape(KSUP, 2, 128, msh).transpose(2, 0, 1, 3).reshape(128, KSUP * 2 * msh)
        ).astype(NP_F8)
        embT = np.ascontiguousarray(emb[k * msh : (k + 1) * msh, :].T).astype(NP_BF16)
        ins.append({"adjx": adjx, "embx": embx, "embT": embT})
    return ins


def make_stage2_inputs(adj, xnT, src, dst_, nc_cores=NC):
    ql = QL
    # xns: (p, j*N + c) = xnT[j*128 + p, c]
    xns = np.ascontiguousarray(xnT.reshape(2, 128, N).transpose(1, 0, 2).reshape(128, 2 * N))
    xnT_f32 = xnT.astype(np.float32)
    ins = []
    for k in range(nc_cores):
        s_k = src[k * ql : (k + 1) * ql]
        d_k = dst_[k * ql : (k + 1) * ql]
        uniq = np.unique(np.concatenate([s_k, d_k]))
        tblW = np.zeros((2 * ql, N), NP_F8)
        tblW[: len(uniq)] = adj[uniq].astype(NP_F8)
        SLW = 2000
        # [NSLAB, r, SLW] column-slab layout for the accumulating gathers
        tblS = np.ascontiguousarray(
            tblW.reshape(2 * ql, N // SLW, SLW).transpose(1, 0, 2)
        )

        def pack_cols(cols):
            # [256, ql] -> (p, j*ql + e)
            g = xnT_f32[:, cols]
            return np.ascontiguousarray(
                g.reshape(2, 128, ql).transpose(1, 0, 2).reshape(128, 2 * ql)
            ).astype(NP_F8)

        ins.append(
            {
                "xns": xns,
                "tblW": tblW,
                "tblS": tblS,
                "idxs": np.searchsorted(uniq, s_k).astype(np.int32)[:, None],
                "idxd": np.searchsorted(uniq, d_k).astype(np.int32)[:, None],
                "ut": pack_cols(s_k),
                "vt": pack_cols(d_k),
            }
        )
    return ins


_progs = {}
LAST_RESULTS = []  # BassKernelResults of the most recent kernel() call (for profiling)


def _get(name, builder):
    if name not in _progs:
        _progs[name] = builder()
    return _progs[name]


def kernel(emb_weight, adj, edges):
    emb = np.asarray(emb_weight, dtype=np.float32)
    adj = np.asarray(adj, dtype=np.float32)
    edges = np.asarray(edges)
    src = edges[0].astype(np.int64)
    dst_ = edges[1].astype(np.int64)

    s1 = _get("s1", build_stage1)
    s2 = _get("s2", build_stage2)

    in1 = make_stage1_inputs(emb, adj)
    r1 = bass_utils.run_bass_kernel_spmd(s1, in1, core_ids=list(range(NC)))
    xnT = np.concatenate(
        [np.asarray(r1.results[k]["xnT"]) for k in range(NC)], axis=1
    )

    in2 = make_stage2_inputs(adj, xnT, src, dst_)
    r2 = bass_utils.run_bass_kernel_spmd(s2, in2, core_ids=list(range(NC)))
    w = np.concatenate([r2.results[k]["w"][:, 0] for k in range(NC)])

    LAST_RESULTS.clear()
    LAST_RESULTS.extend([r1, r2])
    return w.astype(np.float32)

